# revision 1
# baseline (speedup 1.0000x reference)
"""Trainium2 Bass kernel for fused MultiHeadAttention + residual + LayerNorm.

Problem: query [4, 2048, 512] f32, H=8 heads (hd=64), fused QKV projection,
key-padding-mask softmax, attn @ V, residual add, LayerNorm over D=512.

Sharding: 8 cores = 4 batches x 2 query-halves. Each core handles one batch's
full K/V (T=2048) and 1024 query rows, so heads stay local and the output
LayerNorm needs no cross-core communication. K/V projection is duplicated
between the 2 cores sharing a batch (cheap relative to attention).

All matmul operands are bf16 (accumulation fp32 in PSUM) — keeping the PE
stream homogeneous matters: mixing f32r and bf16 matmuls measured 3x slower
per matmul than either alone (weight-path mode thrash). Softmax, residual and
LayerNorm stay fp32.

Per-core flow:
  X^T [512, 2048] bf16 (host-pre-transposed), W^T [512, 1536] bf16
  K^T [512, 2048] bf16, Q^T [512, 1024] bf16 (head-major rows)
  V   16 x [128, 8, 65] bf16  (col 0 of each head group = 1.0 -> denominator)
  S^T [128k, 1024q] f32 PSUM per (head, k-tile) -> ACT exp with per-partition
      mask bias and scale=1/8 -> P^T bf16
  O^T [65, 1024] f32 = [1|V_h].T @ P^T accumulated over k-tiles (V stationary
      so weight loads hide behind the 512-col moving stream)
  PE-transpose [65,128] chunks -> [128,65], then DVE reciprocal(denom) and
      a fused multiply-add folds the residual in per head slice
  LayerNorm: DVE row-sum, ACT Square(y-mean) accum for variance, Sqrt,
      reciprocal, normalize + affine -> DMA out [1024, 512] f32.

Scheduling notes (measured on HW): PE warm-up matmuls run during the initial
DMA wait (HAM clock gate); critical DMAs split across both HWDGE issuing
engines; score matmuls zero-padded to K=128 contraction (HAM ignores K<128);
all projections emitted before any attention@V transpose so the shared psum
pool never cross-serializes; LayerNorm pipelined per q-tile off head 7.
"""

import numpy as np

B, T, D = 4, 2048, 512
H, HD = 8, 64
Q = T // 2          # query rows per core
NCORES = 8
KT = T // 128       # 16 k-tiles
QT = Q // 128       # 8 q-tiles
DC = D // 128       # 4 contraction chunks
SCALE = 1.0 / np.sqrt(HD)  # 0.125
EPS = 1e-5
MASK_BIAS = -1e9

_CACHE = {}


def _emit(nc, tc, tens):
    import contextlib

    import concourse.bass as bass
    from concourse import mybir
    from concourse.masks import make_identity

    f32 = mybir.dt.float32
    bf16 = mybir.dt.bfloat16
    Alu = mybir.AluOpType
    Act = mybir.ActivationFunctionType

    with contextlib.ExitStack() as stack:
        persist = stack.enter_context(tc.tile_pool(name="persist", bufs=1))
        small = stack.enter_context(tc.tile_pool(name="small", bufs=8))
        expp = stack.enter_context(tc.tile_pool(name="expp", bufs=KT + 3))
        otsbp = stack.enter_context(tc.tile_pool(name="otsbp", bufs=2))
        outp = stack.enter_context(tc.tile_pool(name="outp", bufs=2))
        pps = stack.enter_context(tc.tile_pool(name="pps", bufs=2, space="PSUM"))
        stp = stack.enter_context(tc.tile_pool(name="stp", bufs=2, space="PSUM"))
        scr = stack.enter_context(tc.tile_pool(name="scr", bufs=2, space="PSUM"))

        # ---- persistent tiles ----
        wt_sb = [persist.tile([128, 3 * D], bf16, name=f"wtsb{i}", tag=f"wtsb{i}")
                 for i in range(DC)]
        xt_sb = [persist.tile([128, T], bf16, name=f"xtsb{i}", tag=f"xtsb{i}")
                 for i in range(DC)]
        xq_sb = [persist.tile([128, Q], bf16, name=f"xqsb{i}", tag=f"xqsb{i}")
                 for i in range(DC)]
        kt_sb = [persist.tile([128, T], bf16, name=f"ktsb{i}", tag=f"ktsb{i}")
                 for i in range(DC)]
        # Per-head Q^T padded to 128 contraction rows: rows (h%2)*64..+64 hold
        # Q_h, the other 64 rows stay zero. Keeps the score matmuls at K=128 —
        # K=64 matmuls don't register as PE activity for the HAM clock gate
        # and leave the whole attention phase throttled to 1.2 GHz.
        qt_pad = [persist.tile([128, Q], bf16, name=f"qtpad{h}", tag=f"qtpad{h}")
                  for h in range(H)]
        v_sb = [persist.tile([128, H, HD + 1], bf16, name=f"vsb{k}", tag=f"vsb{k}")
                for k in range(KT)]
        oacc = [persist.tile([128, D], f32, name=f"oacc{q}", tag=f"oacc{q}")
                for q in range(QT)]
        xres_sb = persist.tile([128, QT, D], f32, name="xres_sb", tag="xres_sb")
        btr_sb = persist.tile([128, 12], f32, name="btr_sb", tag="btr_sb")
        maskb_sb = persist.tile([128, KT], f32, name="maskb_sb", tag="maskb_sb")
        bvb_sb = persist.tile([128, D], f32, name="bvb_sb", tag="bvb_sb")
        lnw_sb = persist.tile([128, D], f32, name="lnw_sb", tag="lnw_sb")
        lnb_sb = persist.tile([128, D], f32, name="lnb_sb", tag="lnb_sb")
        eps_sb = persist.tile([128, 1], f32, name="eps_sb", tag="eps_sb")
        ident65 = persist.tile([HD + 1, HD + 1], f32, name="ident65",
                               tag="ident65")

        # ---- input DMAs, in dependency-priority order: the first score
        # matmul needs K^T block 0 (wt K-columns + xt t-chunk 0) and Q^T
        # block 0, so those chunks land first ----
        rows = lambda i: slice(i * 128, (i + 1) * 128)
        wm_sb = persist.tile([128, 640], bf16, name="wm_sb", tag="wm_sb")
        nc.vector.memset(wm_sb, 0.5)
        # Critical-path loads (K^T block needs wt K-cols + all of xt) split
        # across the two HWDGE issuing engines so the per-engine semaphore
        # chains run in parallel.
        nc.scalar.dma_start(out=btr_sb, in_=tens["btr"][:])
        nc.scalar.dma_start(out=maskb_sb, in_=tens["maskb"][:])
        for i in range(2):
            nc.sync.dma_start(out=xt_sb[i], in_=tens["xt"][rows(i), :])
            nc.scalar.dma_start(out=xt_sb[i + 2],
                                in_=tens["xt"][rows(i + 2), :])
            nc.sync.dma_start(out=wt_sb[i + 2][:, D:2 * D],
                              in_=tens["wt"][rows(i + 2), D:2 * D])
            nc.scalar.dma_start(out=wt_sb[i][:, D:2 * D],
                              in_=tens["wt"][rows(i), D:2 * D])
        for i in range(2):
            nc.sync.dma_start(out=wt_sb[i][:, 0:D],
                              in_=tens["wt"][rows(i), 0:D])
            nc.scalar.dma_start(out=wt_sb[i + 2][:, 0:D],
                                in_=tens["wt"][rows(i + 2), 0:D])
            nc.sync.dma_start(out=xq_sb[i], in_=tens["xq"][rows(i), :])
            nc.scalar.dma_start(out=xq_sb[i + 2],
                                in_=tens["xq"][rows(i + 2), :])
        for i in range(2):
            nc.sync.dma_start(out=wt_sb[i][:, 2 * D:3 * D],
                              in_=tens["wt"][rows(i), 2 * D:3 * D])
            nc.scalar.dma_start(out=wt_sb[i + 2][:, 2 * D:3 * D],
                                in_=tens["wt"][rows(i + 2), 2 * D:3 * D])

        def bcast_row(dst, src_handle):
            src = src_handle[:]
            ap = bass.AP(tensor=src.tensor, offset=src.offset,
                         ap=[[0, 128]] + list(src.ap))
            nc.sync.dma_start(out=dst, in_=ap)

        bcast_row(bvb_sb, tens["bv"])
        nc.vector.memset(eps_sb, EPS)
        for h in range(H):
            z0 = 64 * (1 - (h % 2))
            (nc.vector if h < 2 else nc.gpsimd).memset(
                qt_pad[h][z0:z0 + HD, :], 0.0)
        for k in range(KT):
            nc.gpsimd.memset(v_sb[k][:, :, 0:1], 1.0)
        make_identity(nc, ident65)

        # ---- PE warm-up: K=128 matmuls with no data deps run during the
        # initial DMA wait so the HAM clock gate is already open (2.4 GHz)
        # when the projections start. The result is never used.
        wmps = stp.tile([128, Q], f32, name="wmps", tag="st")
        for i in range(24):
            nc.tensor.matmul(wmps[:, 0:512], wm_sb[:, 0:128],
                             wm_sb[:, 128:640], start=True, stop=True)
        wm_out = small.tile([128, 1], f32, name="wm_out", tag="wm_out")
        nc.vector.tensor_copy(out=wm_out, in_=wmps[:, 0:1])

        # deferred loads only needed after the first normalize / epilogue
        for q in range(QT):
            nc.gpsimd.dma_start(out=xres_sb[:, q, :],
                                in_=tens["xres"][q * 128:(q + 1) * 128, :])
        for dst, key in ((lnw_sb, "lnw"), (lnb_sb, "lnb")):
            src = tens[key][:]
            ap = bass.AP(tensor=src.tensor, offset=src.offset,
                         ap=[[0, 128]] + list(src.ap))
            nc.gpsimd.dma_start(out=dst, in_=ap)

        # ---- projection emitters ----
        def emit_kt(i):
            for tcn in range(T // 512):
                ps = pps.tile([128, 512], f32, name="kps", tag="pps")
                for dc in range(DC):
                    nc.tensor.matmul(
                        ps, wt_sb[dc][:, D + i * 128: D + (i + 1) * 128],
                        xt_sb[dc][:, tcn * 512:(tcn + 1) * 512],
                        start=(dc == 0), stop=(dc == DC - 1))
                nc.vector.tensor_scalar_add(
                    out=kt_sb[i][:, tcn * 512:(tcn + 1) * 512],
                    in0=ps, scalar1=btr_sb[:, 4 + i:5 + i])

        def emit_qt(i):
            for qcn in range(Q // 512):
                ps = pps.tile([128, 512], f32, name="qps", tag="pps")
                for dc in range(DC):
                    nc.tensor.matmul(
                        ps, wt_sb[dc][:, i * 128:(i + 1) * 128],
                        xq_sb[dc][:, qcn * 512:(qcn + 1) * 512],
                        start=(dc == 0), stop=(dc == DC - 1))
                for j in range(2):
                    r0 = j * HD
                    nc.vector.tensor_scalar_add(
                        out=qt_pad[2 * i + j][r0:r0 + HD,
                                              qcn * 512:(qcn + 1) * 512],
                        in0=ps[r0:r0 + HD, :],
                        scalar1=btr_sb[r0:r0 + HD, i:i + 1])

        def emit_v(k):
            ps = pps.tile([128, 512], f32, name="vps", tag="pps")
            for dc in range(DC):
                nc.tensor.matmul(
                    ps, xt_sb[dc][:, k * 128:(k + 1) * 128],
                    wt_sb[dc][:, 2 * D:3 * D],
                    start=(dc == 0), stop=(dc == DC - 1))
            nc.vector.scalar_tensor_tensor(
                out=v_sb[k][:, :, 1:HD + 1],
                in0=ps.rearrange("p (h d) -> p h d", h=H),
                scalar=1.0,
                in1=bvb_sb.rearrange("p (h d) -> p h d", h=H),
                op0=Alu.mult, op1=Alu.add)

        # ---- residual + LayerNorm emitter (one q-tile) ----
        def emit_ln(q):
            # oacc[q] already holds attention + residual
            rowsum = small.tile([128, 1], f32, name="rowsum", tag="rowsum")
            nc.vector.reduce_sum(out=rowsum, in_=oacc[q],
                                 axis=mybir.AxisListType.X)
            mean = small.tile([128, 1], f32, name="mean", tag="mean")
            nc.vector.tensor_scalar_mul(out=mean, in0=rowsum,
                                        scalar1=1.0 / D)
            negmean = small.tile([128, 1], f32, name="negmean",
                                 tag="negmean")
            nc.vector.tensor_scalar_mul(out=negmean, in0=rowsum,
                                        scalar1=-1.0 / D)
            # D * variance = sum (y - mean)^2, on the otherwise-idle ACT
            vscr = outp.tile([128, D], f32, name="vscr", tag="vscr")
            varsum = small.tile([128, 1], f32, name="varsum", tag="varsum")
            nc.scalar.activation(out=vscr, in_=oacc[q], func=Act.Square,
                                 bias=negmean, accum_out=varsum)
            sd = small.tile([128, 1], f32, name="sd", tag="sd")
            nc.scalar.activation(out=sd, in_=varsum, func=Act.Sqrt,
                                 bias=eps_sb, scale=1.0 / D)
            rstd = small.tile([128, 1], f32, name="rstd", tag="rstd")
            nc.vector.reciprocal(out=rstd, in_=sd)
            yn = outp.tile([128, D], f32, name="yn", tag="yn")
            nc.vector.tensor_scalar(
                out=yn, in0=oacc[q], scalar1=mean, scalar2=rstd,
                op0=Alu.subtract, op1=Alu.mult)
            yw = outp.tile([128, D], f32, name="yw", tag="yw")
            nc.vector.scalar_tensor_tensor(
                out=yw, in0=yn, scalar=1.0, op0=Alu.mult,
                in1=lnw_sb, op1=Alu.mult)
            yo = outp.tile([128, D], f32, name="yo", tag="yo")
            nc.gpsimd.tensor_tensor(out=yo, in0=yw, in1=lnb_sb,
                                    op=Alu.add)
            nc.scalar.dma_start(out=tens["out"][q * 128:(q + 1) * 128, :],
                                in_=yo)

        # ---- attention head emitters (scores/exp vs attention@V) ----
        head_expts = {}

        def emit_scores(h):
            blk = h // 2
            expts = head_expts[h] = []
            for k in range(KT):
                st = stp.tile([128, Q], f32, name="st", tag="st")
                for qcn in range(Q // 512):
                    nc.tensor.matmul(
                        st[:, qcn * 512:(qcn + 1) * 512],
                        kt_sb[blk][:, k * 128:(k + 1) * 128],
                        qt_pad[h][:, qcn * 512:(qcn + 1) * 512],
                        start=None, stop=None)
                e = expp.tile([128, Q], bf16, name="expt", tag="expt")
                nc.scalar.activation(out=e, in_=st, func=Act.Exp,
                                     bias=maskb_sb[:, k:k + 1], scale=SCALE)
                expts.append(e)

        def emit_av(h, inter_with=None):
            expts = head_expts[h]
            # O^T[1+d, q] accumulated over k-tiles; V_h stationary (65 cols)
            # so its weight load hides behind the 512-col moving stream. Two
            # half-tiles (1 PSUM bank each) accumulate in lockstep with the
            # exp stream; optionally interleaved per k-tile with the NEXT
            # head's score/exp emission so ACT never starves behind a dense
            # attention@V block.
            otsb = otsbp.tile([HD + 1, Q], f32, name="otsb", tag="otsb")
            ots = [scr.tile([HD + 1, 512], f32, name=f"ot{qcn}", tag="ot")
                   for qcn in range(Q // 512)]
            if inter_with is not None:
                nblk = inter_with // 2
                nexpts = head_expts[inter_with] = []
            for k in range(KT):
                for qcn in range(Q // 512):
                    nc.tensor.matmul(
                        ots[qcn], v_sb[k][:, h, :],
                        expts[k][:, qcn * 512:(qcn + 1) * 512],
                        start=(k == 0), stop=(k == KT - 1))
                if inter_with is not None:
                    st = stp.tile([128, Q], f32, name="st", tag="st")
                    for qcn in range(Q // 512):
                        nc.tensor.matmul(
                            st[:, qcn * 512:(qcn + 1) * 512],
                            kt_sb[nblk][:, k * 128:(k + 1) * 128],
                            qt_pad[inter_with][:, qcn * 512:(qcn + 1) * 512],
                            start=None, stop=None)
                    e = expp.tile([128, Q], bf16, name="expt", tag="expt")
                    nc.scalar.activation(out=e, in_=st, func=Act.Exp,
                                         bias=maskb_sb[:, k:k + 1],
                                         scale=SCALE)
                    nexpts.append(e)
            for qcn in range(Q // 512):
                nc.vector.tensor_copy(
                    out=otsb[:, qcn * 512:(qcn + 1) * 512], in_=ots[qcn])
            for q in range(QT):
                tp = pps.tile([128, HD + 1], f32, name="tp", tag="pps")
                nc.tensor.transpose(
                    tp, otsb[:, q * 128:(q + 1) * 128], ident65)
                rec = small.tile([128, 1], f32, name="rec", tag="rec")
                nc.vector.reciprocal(out=rec, in_=tp[:, 0:1])
                nc.vector.scalar_tensor_tensor(
                    out=oacc[q][:, h * HD:(h + 1) * HD],
                    in0=tp[:, 1:HD + 1], scalar=rec, op0=Alu.mult,
                    in1=xres_sb[:, q, h * HD:(h + 1) * HD], op1=Alu.add)
                if h == H - 1:
                    emit_ln(q)

        # ---- emission: block-0 projections and head 0's scores first (exp
        # stream starts before V-proj); each attention@V interleaves per
        # k-tile with the next head's scores so ACT stays fed ----
        emit_kt(0)
        emit_qt(0)
        emit_scores(0)
        for k in range(KT):
            emit_v(k)
        emit_av(0, inter_with=1)
        emit_kt(1)
        emit_qt(1)
        emit_av(1, inter_with=2)
        emit_kt(2)
        emit_qt(2)
        emit_av(2, inter_with=3)
        emit_kt(3)
        emit_qt(3)
        for h in range(3, H - 1):
            emit_av(h, inter_with=h + 1)
        emit_av(H - 1)

        # (residual + LayerNorm is emitted per q-tile from the last head)


def _build():
    import concourse.bacc as bacc
    import concourse.tile as tile
    from concourse import mybir

    f32 = mybir.dt.float32
    bf16 = mybir.dt.bfloat16
    nc = bacc.Bacc("TRN2", target_bir_lowering=False, debug=False)

    tens = {
        "xt": nc.dram_tensor("xt", [D, T], bf16, kind="ExternalInput"),
        "xq": nc.dram_tensor("xq", [D, Q], bf16, kind="ExternalInput"),
        "xres": nc.dram_tensor("xres", [Q, D], f32, kind="ExternalInput"),
        "wt": nc.dram_tensor("wt", [D, 3 * D], bf16, kind="ExternalInput"),
        "btr": nc.dram_tensor("btr", [128, 12], f32, kind="ExternalInput"),
        "bv": nc.dram_tensor("bv", [D], f32, kind="ExternalInput"),
        "maskb": nc.dram_tensor("maskb", [128, KT], f32, kind="ExternalInput"),
        "lnw": nc.dram_tensor("lnw", [D], f32, kind="ExternalInput"),
        "lnb": nc.dram_tensor("lnb", [D], f32, kind="ExternalInput"),
        "out": nc.dram_tensor("out", [Q, D], f32, kind="ExternalOutput"),
    }

    with tile.TileContext(nc) as tc:
        _emit(nc, tc, tens)
    nc.compile()
    return nc


def make_in_maps(query, key_mask, in_proj_weight, in_proj_bias, ln_weight,
                 ln_bias):
    import ml_dtypes

    bf = ml_dtypes.bfloat16
    query = np.asarray(query, dtype=np.float32)
    key_mask = np.asarray(key_mask)
    w = np.asarray(in_proj_weight, dtype=np.float32)
    b = np.asarray(in_proj_bias, dtype=np.float32)
    lnw = np.asarray(ln_weight, dtype=np.float32)
    lnb = np.asarray(ln_bias, dtype=np.float32)

    wt = np.ascontiguousarray(w.T).astype(bf)
    btr = np.ascontiguousarray(b.reshape(12, 128).T)
    bv = np.ascontiguousarray(b[2 * D:3 * D])
    in_maps = []
    for c in range(NCORES):
        bi, half = c // 2, c % 2
        xb = query[bi]
        xbt = np.ascontiguousarray(xb.T).astype(bf)
        maskb = np.where(key_mask[bi], np.float32(MASK_BIAS), np.float32(0.0))
        in_maps.append({
            "xt": xbt,
            "xq": np.ascontiguousarray(xbt[:, half * Q:(half + 1) * Q]),
            "xres": np.ascontiguousarray(xb[half * Q:(half + 1) * Q]),
            "wt": wt,
            "btr": btr,
            "bv": bv,
            "maskb": np.ascontiguousarray(
                maskb.astype(np.float32).reshape(KT, 128).T),
            "lnw": lnw,
            "lnb": lnb,
        })
    return in_maps


def assemble(results):
    out = np.empty((B, T, D), dtype=np.float32)
    for c in range(NCORES):
        bi, half = c // 2, c % 2
        out[bi, half * Q:(half + 1) * Q] = results[c]["out"]
    return out


def get_nc():
    if "nc" not in _CACHE:
        _CACHE["nc"] = _build()
    return _CACHE["nc"]


def kernel(query, key_mask, in_proj_weight, in_proj_bias, ln_weight, ln_bias):
    from concourse.bass_utils import run_bass_kernel_spmd

    nc = get_nc()
    in_maps = make_in_maps(query, key_mask, in_proj_weight, in_proj_bias,
                           ln_weight, ln_bias)
    res = run_bass_kernel_spmd(nc, in_maps, core_ids=list(range(NCORES)))
    return assemble(res.results)



# revision 12
# speedup vs baseline: 1.0862x; 1.0862x over previous
"""Trainium2 Bass kernel for fused MultiHeadAttention + residual + LayerNorm.

Problem: query [4, 2048, 512] f32, H=8 heads (hd=64), fused QKV projection,
key-padding-mask softmax, attn @ V, residual add, LayerNorm over D=512.

Sharding: 8 cores = 4 batches x 2 query-halves. Each core handles one batch's
full K/V (T=2048) and 1024 query rows, so heads stay local and the output
LayerNorm needs no cross-core communication. K/V projection is duplicated
between the 2 cores sharing a batch (cheap relative to attention).

The kernel is ACT-bound: softmax exp is H*T*Q = 16.8M elements per core and
the activation engine runs 1 elem/lane/cycle at 1.2 GHz -> ~133 us minimum.
Everything else is scheduled around keeping the ACT exp stream gap-free:

  - projections + scores stay bf16 (fp8 Q/K measured 2.6e-2 rel err - fails);
    score matmuls zero-padded to K=128 contraction (HAM ignores K<128).
  - attn@V runs fp8e4 DoubleRow: V pairs [128, 2, h, 65] and exp-output
    pairs [128, 2, 1024] pack two k-tiles per pass at 0.5 cycles/col,
    cutting the AV matmul stream 4x vs bf16. exp output carries a -4 logit
    shift (folded into the mask bias) so p <= ~7.4 stays in e4m3 range;
    numerator and denominator (ones column in V) scale identically so the
    softmax ratio is unchanged.
  - input DMAs are split fine-grained across all four issuing queues
    (sync/scalar/vector/gpsimd) with the head-0 critical path (xq, wt q/k
    block-0 columns, xt t-chunk 0) first, so the first exp lands ~10 us in.
  - per head-step the PE emits scores for head h+1 in 4-k-tile slices with
    AV/projection/V work inserted between slices, keeping every insert under
    the ~2-tile PSUM backlog the exp stream can absorb without starving.
  - LayerNorm tail per q-tile is engine-split (Pool rowsum, ACT sum-of-
    squares via E[y^2], DVE normalize) and pipelined off head 7.
"""

import numpy as np

B, T, D = 4, 2048, 512
H, HD = 8, 64
Q = T // 2          # query rows per core
NCORES = 8
KT = T // 128       # 16 k-tiles
KP = KT // 2        # 8 k-tile pairs
QT = Q // 128       # 8 q-tiles
DC = D // 128       # 4 contraction chunks
SCALE = 1.0 / np.sqrt(HD)  # 0.125
EPS = 1e-5
MASK_BIAS = -1e9
SHIFT = -4.0        # logit shift so exp fits fp8e4 range

_CACHE = {}


def _emit(nc, tc, tens, trivial_ln):
    import contextlib

    import concourse.bass as bass
    from concourse import mybir
    from concourse.masks import make_identity

    f32 = mybir.dt.float32
    bf16 = mybir.dt.bfloat16
    fp8 = mybir.dt.float8e4
    Alu = mybir.AluOpType
    Act = mybir.ActivationFunctionType
    DR = mybir.MatmulPerfMode.DoubleRow

    with contextlib.ExitStack() as stack:
        persist = stack.enter_context(tc.tile_pool(name="persist", bufs=1))
        small = stack.enter_context(tc.tile_pool(name="small", bufs=12))
        expp = stack.enter_context(tc.tile_pool(name="expp", bufs=22))
        otsbp = stack.enter_context(tc.tile_pool(name="otsbp", bufs=2))
        outp = stack.enter_context(tc.tile_pool(name="outp", bufs=4))
        pps = stack.enter_context(tc.tile_pool(name="pps", bufs=2, space="PSUM"))
        stp = stack.enter_context(tc.tile_pool(name="stp", bufs=2, space="PSUM"))
        scr = stack.enter_context(tc.tile_pool(name="scr", bufs=2, space="PSUM"))

        # ---- persistent tiles ----
        wt_sb = [persist.tile([128, 3 * D], bf16, name=f"wtsb{i}", tag=f"wtsb{i}")
                 for i in range(DC)]
        xt_sb = [persist.tile([128, T], bf16, name=f"xtsb{i}", tag=f"xtsb{i}")
                 for i in range(DC)]
        xq_sb = [persist.tile([128, Q], bf16, name=f"xqsb{i}", tag=f"xqsb{i}")
                 for i in range(DC)]
        kt_sb = [persist.tile([128, T], bf16, name=f"ktsb{i}", tag=f"ktsb{i}")
                 for i in range(DC)]
        # Per-head Q^T padded to 128 contraction rows: rows (h%2)*64..+64 hold
        # Q_h, the other 64 rows stay zero (K=128 keeps the HAM clock gate
        # open; K=64 matmuls throttle the whole attention phase).
        qt_pad = [persist.tile([128, Q], bf16, name=f"qtpad{h}", tag=f"qtpad{h}")
                  for h in range(H)]
        # V k-tile pairs for fp8 DoubleRow attn@V. DoubleRow stationary
        # width must be 32/64/128, so each head's stationary is
        # [V_h (64 cols) | ones (64 cols)]: output rows 0..63 are O^T and
        # rows 64.. replicate the softmax denominator, at no extra cost
        # (matmul time depends only on moving columns).
        v8 = [persist.tile([128, H, 2, 128], fp8, name=f"v8_{p}",
                           tag=f"v8_{p}") for p in range(KP)]
        oacc = [persist.tile([128, D], f32, name=f"oacc{q}", tag=f"oacc{q}")
                for q in range(QT)]
        xres_sb = persist.tile([128, QT, D], f32, name="xres_sb", tag="xres_sb")
        btr_sb = persist.tile([128, 12], f32, name="btr_sb", tag="btr_sb")
        maskb_sb = persist.tile([128, KT], f32, name="maskb_sb", tag="maskb_sb")
        bvb_sb = persist.tile([128, D], f32, name="bvb_sb", tag="bvb_sb")
        lnw_sb = persist.tile([128, D], f32, name="lnw_sb", tag="lnw_sb")
        lnb_sb = persist.tile([128, D], f32, name="lnb_sb", tag="lnb_sb")
        eps_sb = persist.tile([128, 1], f32, name="eps_sb", tag="eps_sb")
        ident65 = persist.tile([HD + 1, HD + 1], f32, name="ident65",
                               tag="ident65")

        # ---- input DMAs. Critical path = everything head-0's first exp
        # needs (xq full, wt q-block0 + k-block0 columns, xt t-chunk 0),
        # split fine and round-robined over all four issuing queues. ----
        engs = [nc.sync, nc.scalar, nc.gpsimd, nc.sync]

        nc.scalar.dma_start(out=btr_sb, in_=tens["btr"][:])
        nc.sync.dma_start(out=maskb_sb, in_=tens["maskb"][:])
        rows = lambda i: slice(i * 128, (i + 1) * 128)
        for i in range(DC):
            engs[i].dma_start(out=xq_sb[i], in_=tens["xq"][rows(i), :])
        for i in range(DC):
            engs[(i + 1) % 4].dma_start(out=wt_sb[i][:, 0:128],
                                        in_=tens["wt"][rows(i), 0:128])
            engs[(i + 2) % 4].dma_start(out=wt_sb[i][:, D:D + 128],
                                        in_=tens["wt"][rows(i), D:D + 128])
        for i in range(DC):
            engs[(i + 3) % 4].dma_start(out=xt_sb[i][:, 0:512],
                                        in_=tens["xt"][rows(i), 0:512])
        # second wave: rest of xt (k-tiles 4..15), V cols, remaining K/Q cols
        for i in range(DC):
            engs[i].dma_start(out=xt_sb[i][:, 512:1280],
                              in_=tens["xt"][rows(i), 512:1280])
            engs[(i + 2) % 4].dma_start(out=xt_sb[i][:, 1280:2048],
                                        in_=tens["xt"][rows(i), 1280:2048])
        for i in range(DC):
            engs[(i + 1) % 4].dma_start(out=wt_sb[i][:, 2 * D:3 * D],
                                        in_=tens["wt"][rows(i), 2 * D:3 * D])
        for i in range(DC):
            engs[(i + 3) % 4].dma_start(out=wt_sb[i][:, D + 128:2 * D],
                                        in_=tens["wt"][rows(i), D + 128:2 * D])
            engs[i].dma_start(out=wt_sb[i][:, 128:D],
                              in_=tens["wt"][rows(i), 128:D])

        def bcast_row(dst, src_handle, eng=None):
            src = src_handle[:]
            ap = bass.AP(tensor=src.tensor, offset=src.offset,
                         ap=[[0, 128]] + list(src.ap))
            (eng or nc.sync).dma_start(out=dst, in_=ap)

        bcast_row(bvb_sb, tens["bv"])
        nc.vector.memset(eps_sb, EPS)
        for h in range(H):
            z0 = 64 * (1 - (h % 2))
            (nc.vector if h < 2 else nc.gpsimd).memset(
                qt_pad[h][z0:z0 + HD, :], 0.0)
        for p in range(KP):
            for i in range(2):
                nc.gpsimd.memset(v8[p][:, :, i, HD:128], 1.0)
        make_identity(nc, ident65)

        # ---- PE warm-up: K=128 matmuls with no data deps run during the
        # initial DMA wait so the HAM clock gate is already open when the
        # projections start. The result is never used.
        wm_sb = persist.tile([128, 640], bf16, name="wm_sb", tag="wm_sb")
        nc.vector.memset(wm_sb, 0.5)
        wmps = stp.tile([128, Q], f32, name="wmps", tag="st")
        for i in range(20):
            nc.tensor.matmul(wmps[:, 0:512], wm_sb[:, 0:128],
                             wm_sb[:, 128:640], start=True, stop=True)
        wm_out = small.tile([128, 1], f32, name="wm_out", tag="wm_out")
        nc.vector.tensor_copy(out=wm_out, in_=wmps[:, 0:1])

        # deferred loads only needed after the first normalize / epilogue
        for q in range(QT):
            nc.gpsimd.dma_start(out=xres_sb[:, q, :],
                                in_=tens["xres"][q * 128:(q + 1) * 128, :])
        bcast_row(lnw_sb, tens["lnw"], nc.gpsimd)
        bcast_row(lnb_sb, tens["lnb"], nc.gpsimd)

        # ---- projection emitters ----
        def emit_kt(i, tcns):
            for tcn in tcns:
                ps = pps.tile([128, 512], f32, name="kps", tag="pps")
                for dc in range(DC):
                    nc.tensor.matmul(
                        ps, wt_sb[dc][:, D + i * 128: D + (i + 1) * 128],
                        xt_sb[dc][:, tcn * 512:(tcn + 1) * 512],
                        start=(dc == 0), stop=(dc == DC - 1))
                nc.vector.tensor_scalar_add(
                    out=kt_sb[i][:, tcn * 512:(tcn + 1) * 512],
                    in0=ps, scalar1=btr_sb[:, 4 + i:5 + i])

        def emit_qt(i):
            for qcn in range(Q // 512):
                ps = pps.tile([128, 512], f32, name="qps", tag="pps")
                for dc in range(DC):
                    nc.tensor.matmul(
                        ps, wt_sb[dc][:, i * 128:(i + 1) * 128],
                        xq_sb[dc][:, qcn * 512:(qcn + 1) * 512],
                        start=(dc == 0), stop=(dc == DC - 1))
                for j in range(2):
                    r0 = j * HD
                    nc.vector.tensor_scalar_add(
                        out=qt_pad[2 * i + j][r0:r0 + HD,
                                              qcn * 512:(qcn + 1) * 512],
                        in0=ps[r0:r0 + HD, :],
                        scalar1=btr_sb[r0:r0 + HD, i:i + 1])

        def emit_v(ks):
            for k in ks:
                ps = pps.tile([128, 512], f32, name="vps", tag="pps")
                for dc in range(DC):
                    nc.tensor.matmul(
                        ps, xt_sb[dc][:, k * 128:(k + 1) * 128],
                        wt_sb[dc][:, 2 * D:3 * D],
                        start=(dc == 0), stop=(dc == DC - 1))
                nc.vector.scalar_tensor_tensor(
                    out=v8[k // 2][:, :, k % 2, 0:HD],
                    in0=ps.rearrange("p (h d) -> p h d", h=H),
                    scalar=1.0,
                    in1=bvb_sb.rearrange("p (h d) -> p h d", h=H),
                    op0=Alu.mult, op1=Alu.add)

        # ---- residual + LayerNorm emitter (one q-tile) ----
        # var from E[y^2] - mean^2 so the row-sum (Pool) and sum-of-squares
        # (ACT) run concurrently; ACT is free by the time the tail runs.
        def emit_ln(q):
            rowsum = small.tile([128, 1], f32, name="rowsum", tag="rowsum")
            cpscr = outp.tile([128, D], f32, name="cpscr", tag="cpscr")
            nc.scalar.activation(out=cpscr, in_=oacc[q], func=Act.Copy,
                                 accum_out=rowsum)
            sqscr = outp.tile([128, D], f32, name="sqscr", tag="sqscr")
            sumsq = small.tile([128, 1], f32, name="sumsq", tag="sumsq")
            nc.scalar.activation(out=sqscr, in_=oacc[q], func=Act.Square,
                                 accum_out=sumsq)
            mean = small.tile([128, 1], f32, name="mean", tag="mean")
            nc.vector.tensor_scalar_mul(out=mean, in0=rowsum,
                                        scalar1=1.0 / D)
            r2 = small.tile([128, 1], f32, name="r2", tag="r2")
            nc.vector.tensor_tensor(out=r2, in0=rowsum, in1=rowsum,
                                    op=Alu.mult)
            # varD = sumsq - rowsum^2/D
            varD = small.tile([128, 1], f32, name="varD", tag="varD")
            nc.vector.scalar_tensor_tensor(
                out=varD, in0=r2, scalar=-1.0 / D, op0=Alu.mult,
                in1=sumsq, op1=Alu.add)
            sd = small.tile([128, 1], f32, name="sd", tag="sd")
            nc.scalar.activation(out=sd, in_=varD, func=Act.Sqrt,
                                 bias=eps_sb, scale=1.0 / D)
            rstd = small.tile([128, 1], f32, name="rstd", tag="rstd")
            nc.vector.reciprocal(out=rstd, in_=sd)
            yn = outp.tile([128, D], f32, name="yn", tag="yn")
            nc.vector.tensor_scalar(
                out=yn, in0=oacc[q], scalar1=mean, scalar2=rstd,
                op0=Alu.subtract, op1=Alu.mult)
            if trivial_ln:
                nc.scalar.dma_start(out=tens["out"][q * 128:(q + 1) * 128, :],
                                    in_=yn)
            else:
                yw = outp.tile([128, D], f32, name="yw", tag="yw")
                nc.vector.scalar_tensor_tensor(
                    out=yw, in0=yn, scalar=1.0, op0=Alu.mult,
                    in1=lnw_sb, op1=Alu.mult)
                yo = outp.tile([128, D], f32, name="yo", tag="yo")
                nc.gpsimd.tensor_tensor(out=yo, in0=yw, in1=lnb_sb,
                                        op=Alu.add)
                nc.scalar.dma_start(out=tens["out"][q * 128:(q + 1) * 128, :],
                                    in_=yo)

        # ---- attention emitters ----
        head_pairs = {}

        def emit_scores(h, ks):
            blk = h // 2
            pairs = head_pairs.setdefault(h, {})
            for k in ks:
                st = stp.tile([128, Q], f32, name="st", tag="st")
                for qcn in range(Q // 512):
                    nc.tensor.matmul(
                        st[:, qcn * 512:(qcn + 1) * 512],
                        kt_sb[blk][:, k * 128:(k + 1) * 128],
                        qt_pad[h][:, qcn * 512:(qcn + 1) * 512],
                        start=None, stop=None)
                if k % 2 == 0:
                    pairs[k // 2] = expp.tile([128, 2, 2, 512], fp8,
                                              name="e8", tag="e8")
                nc.scalar.activation(out=pairs[k // 2][:, :, k % 2, :],
                                     in_=st, func=Act.Exp,
                                     bias=maskb_sb[:, k:k + 1], scale=SCALE)

        def emit_av(h):
            pairs = head_pairs[h]
            ots = [scr.tile([128, 512], f32, name=f"ot{qcn}", tag="ot")
                   for qcn in range(Q // 512)]
            for p in range(KP):
                for qcn in range(Q // 512):
                    nc.tensor.matmul(
                        ots[qcn], v8[p][:, h, :, :],
                        pairs[p][:, qcn, :, :],
                        start=(p == 0), stop=(p == KP - 1),
                        perf_mode=DR)

            otsb = otsbp.tile([HD + 1, Q], f32, name="otsb", tag="otsb")
            for qcn in range(Q // 512):
                nc.vector.tensor_copy(
                    out=otsb[:, qcn * 512:(qcn + 1) * 512],
                    in_=ots[qcn][0:HD + 1, :])
            for q in range(QT):
                tp = pps.tile([128, HD + 1], f32, name="tp", tag="pps")
                nc.tensor.transpose(
                    tp, otsb[:, q * 128:(q + 1) * 128], ident65)
                rec = small.tile([128, 1], f32, name="rec", tag="rec")
                nc.vector.reciprocal(out=rec, in_=tp[:, HD:HD + 1])
                nc.vector.scalar_tensor_tensor(
                    out=oacc[q][:, h * HD:(h + 1) * HD],
                    in0=tp[:, 0:HD], scalar=rec, op0=Alu.mult,
                    in1=xres_sb[:, q, h * HD:(h + 1) * HD], op1=Alu.add)
                if h == H - 1:
                    emit_ln(q)

        # ---- emission schedule. scores h feed the ACT exp stream; every
        # other PE phase is inserted between 4-k-tile score slices in
        # ~2.5us chunks so the 2-tile PSUM backlog keeps ACT from starving.
        # AV for head j runs at step j+2 (after exp j is long done); every
        # projection block lands the step before its first reader. ----
        noop = lambda: None
        slots = {
            0: [lambda: emit_kt(0, [1]), lambda: emit_kt(0, [2, 3]),
                lambda: emit_v(range(0, 3)), lambda: emit_v(range(3, 6))],
            1: [lambda: emit_qt(1), lambda: emit_v(range(6, 9)),
                lambda: emit_v(range(9, 12)),
                lambda: (emit_v(range(12, 16)), emit_kt(1, [0]))],
            2: [lambda: emit_kt(1, [1, 2]), lambda: emit_kt(1, [3]),
                lambda: emit_kt(2, [0, 1]), lambda: emit_kt(2, [2, 3])],
            3: [lambda: emit_qt(2), noop,
                lambda: emit_kt(3, [0, 1]), lambda: emit_kt(3, [2, 3])],
            4: [lambda: emit_qt(3), noop, noop, noop],
            5: [noop, noop, noop, noop],
            6: [noop, noop, noop, noop],
            7: [noop, noop, noop, noop],
        }
        emit_kt(0, [0])
        emit_qt(0)
        for h in range(H):
            ins = slots[h]
            emit_scores(h, range(0, 4))
            ins[0]()
            if h >= 2:
                emit_av(h - 2)
            emit_scores(h, range(4, 8))
            ins[1]()
            emit_scores(h, range(8, 12))
            ins[2]()
            emit_scores(h, range(12, 16))
            ins[3]()
            if h == H - 1:
                emit_av(h - 1)
                emit_av(h)


def _build(trivial_ln):
    import concourse.bacc as bacc
    import concourse.tile as tile
    from concourse import mybir

    f32 = mybir.dt.float32
    bf16 = mybir.dt.bfloat16
    nc = bacc.Bacc("TRN2", target_bir_lowering=False, debug=False)

    tens = {
        "xt": nc.dram_tensor("xt", [D, T], bf16, kind="ExternalInput"),
        "xq": nc.dram_tensor("xq", [D, Q], bf16, kind="ExternalInput"),
        "xres": nc.dram_tensor("xres", [Q, D], f32, kind="ExternalInput"),
        "wt": nc.dram_tensor("wt", [D, 3 * D], bf16, kind="ExternalInput"),
        "btr": nc.dram_tensor("btr", [128, 12], f32, kind="ExternalInput"),
        "bv": nc.dram_tensor("bv", [D], f32, kind="ExternalInput"),
        "maskb": nc.dram_tensor("maskb", [128, KT], f32, kind="ExternalInput"),
        "lnw": nc.dram_tensor("lnw", [D], f32, kind="ExternalInput"),
        "lnb": nc.dram_tensor("lnb", [D], f32, kind="ExternalInput"),
        "out": nc.dram_tensor("out", [Q, D], f32, kind="ExternalOutput"),
    }

    with tile.TileContext(nc) as tc:
        _emit(nc, tc, tens, trivial_ln)
    nc.compile()
    return nc


def make_in_maps(query, key_mask, in_proj_weight, in_proj_bias, ln_weight,
                 ln_bias):
    import ml_dtypes

    bf = ml_dtypes.bfloat16
    query = np.asarray(query, dtype=np.float32)
    key_mask = np.asarray(key_mask)
    w = np.asarray(in_proj_weight, dtype=np.float32)
    b = np.asarray(in_proj_bias, dtype=np.float32)
    lnw = np.asarray(ln_weight, dtype=np.float32)
    lnb = np.asarray(ln_bias, dtype=np.float32)

    wt = np.ascontiguousarray(w.T).astype(bf)
    btr = np.ascontiguousarray(b.reshape(12, 128).T)
    bv = np.ascontiguousarray(b[2 * D:3 * D])
    in_maps = []
    for c in range(NCORES):
        bi, half = c // 2, c % 2
        xb = query[bi]
        xbt = np.ascontiguousarray(xb.T).astype(bf)
        maskb = np.where(key_mask[bi], np.float32(MASK_BIAS),
                         np.float32(SHIFT))
        in_maps.append({
            "xt": xbt,
            "xq": np.ascontiguousarray(xbt[:, half * Q:(half + 1) * Q]),
            "xres": np.ascontiguousarray(xb[half * Q:(half + 1) * Q]),
            "wt": wt,
            "btr": btr,
            "bv": bv,
            "maskb": np.ascontiguousarray(
                maskb.astype(np.float32).reshape(KT, 128).T),
            "lnw": lnw,
            "lnb": lnb,
        })
    return in_maps


def assemble(results):
    out = np.empty((B, T, D), dtype=np.float32)
    for c in range(NCORES):
        bi, half = c // 2, c % 2
        out[bi, half * Q:(half + 1) * Q] = results[c]["out"]
    return out


def get_nc(trivial_ln=True):
    key = ("nc", trivial_ln)
    if key not in _CACHE:
        _CACHE[key] = _build(trivial_ln)
    return _CACHE[key]


def kernel(query, key_mask, in_proj_weight, in_proj_bias, ln_weight, ln_bias):
    from concourse.bass_utils import run_bass_kernel_spmd

    trivial = (np.allclose(np.asarray(ln_weight), 1.0)
               and np.allclose(np.asarray(ln_bias), 0.0))
    nc = get_nc(trivial)
    in_maps = make_in_maps(query, key_mask, in_proj_weight, in_proj_bias,
                           ln_weight, ln_bias)
    res = run_bass_kernel_spmd(nc, in_maps, core_ids=list(range(NCORES)))
    return assemble(res.results)


# revision 14
# speedup vs baseline: 1.0973x; 1.0103x over previous
"""Trainium2 Bass kernel for fused MultiHeadAttention + residual + LayerNorm.

Problem: query [4, 2048, 512] f32, H=8 heads (hd=64), fused QKV projection,
key-padding-mask softmax, attn @ V, residual add, LayerNorm over D=512.

Sharding: 8 cores = 4 batches x 2 query-halves. Each core handles one batch's
full K/V (T=2048) and 1024 query rows, so heads stay local and the output
LayerNorm needs no cross-core communication. K/V projection is duplicated
between the 2 cores sharing a batch (cheap relative to attention).

The kernel is ACT-bound: softmax exp is H*T*Q = 16.8M elements per core and
the activation engine runs 1 elem/lane/cycle at 1.2 GHz -> ~133 us minimum.
Everything else is scheduled around keeping the ACT exp stream gap-free:

  - projections + scores stay bf16 (fp8 Q/K measured 2.6e-2 rel err - fails);
    score matmuls zero-padded to K=128 contraction (HAM ignores K<128).
  - attn@V runs fp8e4 DoubleRow: V pairs [128, 2, h, 65] and exp-output
    pairs [128, 2, 1024] pack two k-tiles per pass at 0.5 cycles/col,
    cutting the AV matmul stream 4x vs bf16. exp output carries a -4 logit
    shift (folded into the mask bias) so p <= ~7.4 stays in e4m3 range;
    numerator and denominator (ones column in V) scale identically so the
    softmax ratio is unchanged.
  - input DMAs are split fine-grained across all four issuing queues
    (sync/scalar/vector/gpsimd) with the head-0 critical path (xq, wt q/k
    block-0 columns, xt t-chunk 0) first, so the first exp lands ~10 us in.
  - per head-step the PE emits scores for head h+1 in 4-k-tile slices with
    AV/projection/V work inserted between slices, keeping every insert under
    the ~2-tile PSUM backlog the exp stream can absorb without starving.
  - LayerNorm tail per q-tile is engine-split (Pool rowsum, ACT sum-of-
    squares via E[y^2], DVE normalize) and pipelined off head 7.
"""

import numpy as np

B, T, D = 4, 2048, 512
H, HD = 8, 64
Q = T // 2          # query rows per core
NCORES = 8
KT = T // 128       # 16 k-tiles
KP = KT // 2        # 8 k-tile pairs
QT = Q // 128       # 8 q-tiles
DC = D // 128       # 4 contraction chunks
SCALE = 1.0 / np.sqrt(HD)  # 0.125
EPS = 1e-5
MASK_BIAS = -1e9
SHIFT = -4.0        # logit shift so exp fits fp8e4 range

_CACHE = {}


def _emit(nc, tc, tens, trivial_ln, trivial_bias):
    import contextlib

    import concourse.bass as bass
    from concourse import mybir
    from concourse.masks import make_identity

    f32 = mybir.dt.float32
    bf16 = mybir.dt.bfloat16
    fp8 = mybir.dt.float8e4
    Alu = mybir.AluOpType
    Act = mybir.ActivationFunctionType
    DR = mybir.MatmulPerfMode.DoubleRow

    with contextlib.ExitStack() as stack:
        persist = stack.enter_context(tc.tile_pool(name="persist", bufs=1))
        small = stack.enter_context(tc.tile_pool(name="small", bufs=12))
        expp = stack.enter_context(tc.tile_pool(name="expp", bufs=22))
        otsbp = stack.enter_context(tc.tile_pool(name="otsbp", bufs=2))
        outp = stack.enter_context(tc.tile_pool(name="outp", bufs=4))
        pps = stack.enter_context(tc.tile_pool(name="pps", bufs=2, space="PSUM"))
        stp = stack.enter_context(tc.tile_pool(name="stp", bufs=2, space="PSUM"))
        scr = stack.enter_context(tc.tile_pool(name="scr", bufs=2, space="PSUM"))

        # ---- persistent tiles ----
        wt_sb = [persist.tile([128, 3 * D], bf16, name=f"wtsb{i}", tag=f"wtsb{i}")
                 for i in range(DC)]
        xt_sb = [persist.tile([128, T], bf16, name=f"xtsb{i}", tag=f"xtsb{i}")
                 for i in range(DC)]
        xq_sb = [persist.tile([128, Q], bf16, name=f"xqsb{i}", tag=f"xqsb{i}")
                 for i in range(DC)]
        kt_sb = [persist.tile([128, T], bf16, name=f"ktsb{i}", tag=f"ktsb{i}")
                 for i in range(DC)]
        # Per-head Q^T padded to 128 contraction rows: rows (h%2)*64..+64 hold
        # Q_h, the other 64 rows stay zero (K=128 keeps the HAM clock gate
        # open; K=64 matmuls throttle the whole attention phase).
        qt_pad = [persist.tile([128, Q], bf16, name=f"qtpad{h}", tag=f"qtpad{h}")
                  for h in range(H)]
        # V k-tile pairs for fp8 DoubleRow attn@V. DoubleRow stationary
        # width must be 32/64/128, so each head's stationary is
        # [V_h (64 cols) | ones (64 cols)]: output rows 0..63 are O^T and
        # rows 64.. replicate the softmax denominator, at no extra cost
        # (matmul time depends only on moving columns).
        v8 = [persist.tile([128, H, 2, 128], fp8, name=f"v8_{p}",
                           tag=f"v8_{p}") for p in range(KP)]
        oacc = [persist.tile([128, D], f32, name=f"oacc{q}", tag=f"oacc{q}")
                for q in range(QT)]
        xres_sb = persist.tile([128, QT, D], f32, name="xres_sb", tag="xres_sb")
        btr_sb = persist.tile([128, 12], f32, name="btr_sb", tag="btr_sb")
        maskb_sb = persist.tile([128, KT], f32, name="maskb_sb", tag="maskb_sb")
        bvb_sb = persist.tile([128, D], f32, name="bvb_sb", tag="bvb_sb")
        lnw_sb = persist.tile([128, D], f32, name="lnw_sb", tag="lnw_sb")
        lnb_sb = persist.tile([128, D], f32, name="lnb_sb", tag="lnb_sb")
        eps_sb = persist.tile([128, 1], f32, name="eps_sb", tag="eps_sb")
        ident65 = persist.tile([HD + 1, HD + 1], f32, name="ident65",
                               tag="ident65")

        # ---- input DMAs. Critical path = everything head-0's first exp
        # needs (xq full, wt q-block0 + k-block0 columns, xt t-chunk 0),
        # split fine and round-robined over all four issuing queues. ----
        engs = [nc.sync, nc.scalar, nc.gpsimd, nc.sync]

        if not trivial_bias:
            nc.scalar.dma_start(out=btr_sb, in_=tens["btr"][:])
        nc.sync.dma_start(out=maskb_sb, in_=tens["maskb"][:])
        rows = lambda i: slice(i * 128, (i + 1) * 128)
        for i in range(DC):
            engs[i].dma_start(out=wt_sb[i][:, D:D + 128],
                              in_=tens["wt"][rows(i), D:D + 128])
            engs[(i + 1) % 4].dma_start(out=xt_sb[i][:, 0:512],
                                        in_=tens["xt"][rows(i), 0:512])
        for i in range(DC):
            engs[(i + 2) % 4].dma_start(out=wt_sb[i][:, 0:128],
                                        in_=tens["wt"][rows(i), 0:128])
            engs[(i + 3) % 4].dma_start(out=xq_sb[i], in_=tens["xq"][rows(i), :])
        # second wave: rest of xt (k-tiles 4..15), V cols, remaining K/Q cols
        for i in range(DC):
            engs[i].dma_start(out=xt_sb[i][:, 512:1280],
                              in_=tens["xt"][rows(i), 512:1280])
            engs[(i + 2) % 4].dma_start(out=xt_sb[i][:, 1280:2048],
                                        in_=tens["xt"][rows(i), 1280:2048])
        for i in range(DC):
            engs[(i + 1) % 4].dma_start(out=wt_sb[i][:, 2 * D:3 * D],
                                        in_=tens["wt"][rows(i), 2 * D:3 * D])
        for i in range(DC):
            engs[(i + 3) % 4].dma_start(out=wt_sb[i][:, D + 128:2 * D],
                                        in_=tens["wt"][rows(i), D + 128:2 * D])
            engs[i].dma_start(out=wt_sb[i][:, 128:D],
                              in_=tens["wt"][rows(i), 128:D])

        def bcast_row(dst, src_handle, eng=None):
            src = src_handle[:]
            ap = bass.AP(tensor=src.tensor, offset=src.offset,
                         ap=[[0, 128]] + list(src.ap))
            (eng or nc.sync).dma_start(out=dst, in_=ap)

        # ---- PE warm-up: K=128 matmuls with no data deps run during the
        # initial DMA wait so the HAM clock gate is already open when the
        # projections start. The result is never used. wm memset is the
        # very first Vector op so the warm-up starts immediately.
        wm_sb = persist.tile([128, 640], bf16, name="wm_sb", tag="wm_sb")
        nc.vector.memset(wm_sb, 0.5)
        wmps = stp.tile([128, Q], f32, name="wmps", tag="st")
        for i in range(20):
            nc.tensor.matmul(wmps[:, 0:512], wm_sb[:, 0:128],
                             wm_sb[:, 128:640], start=True, stop=True)
        wm_out = small.tile([128, 1], f32, name="wm_out", tag="wm_out")
        nc.vector.tensor_copy(out=wm_out, in_=wmps[:, 0:1])

        bcast_row(bvb_sb, tens["bv"])
        nc.vector.memset(eps_sb, EPS)
        for h in range(H):
            z0 = 64 * (1 - (h % 2))
            (nc.vector if h < 2 else nc.gpsimd).memset(
                qt_pad[h][z0:z0 + HD, :], 0.0)
        for p in range(KP):
            for i in range(2):
                nc.gpsimd.memset(v8[p][:, :, i, HD:128], 1.0)
        make_identity(nc, ident65)

        # deferred loads only needed after the first normalize / epilogue
        for q in range(QT):
            nc.gpsimd.dma_start(out=xres_sb[:, q, :],
                                in_=tens["xres"][q * 128:(q + 1) * 128, :])
        bcast_row(lnw_sb, tens["lnw"], nc.gpsimd)
        bcast_row(lnb_sb, tens["lnb"], nc.gpsimd)

        # ---- projection emitters ----
        def emit_kt(i, tcns):
            for tcn in tcns:
                ps = pps.tile([128, 512], f32, name="kps", tag="pps")
                for dc in range(DC):
                    nc.tensor.matmul(
                        ps, wt_sb[dc][:, D + i * 128: D + (i + 1) * 128],
                        xt_sb[dc][:, tcn * 512:(tcn + 1) * 512],
                        start=(dc == 0), stop=(dc == DC - 1))
                if trivial_bias:
                    nc.vector.tensor_copy(
                        out=kt_sb[i][:, tcn * 512:(tcn + 1) * 512], in_=ps)
                else:
                    nc.vector.tensor_scalar_add(
                        out=kt_sb[i][:, tcn * 512:(tcn + 1) * 512],
                        in0=ps, scalar1=btr_sb[:, 4 + i:5 + i])

        def emit_qt(i):
            # write head 2i's rows for both q-halves before head 2i+1's so
            # head 2i's first scores (and the exp stream) start earlier.
            pss = []
            for qcn in range(Q // 512):
                ps = pps.tile([128, 512], f32, name="qps", tag="pps")
                for dc in range(DC):
                    nc.tensor.matmul(
                        ps, wt_sb[dc][:, i * 128:(i + 1) * 128],
                        xq_sb[dc][:, qcn * 512:(qcn + 1) * 512],
                        start=(dc == 0), stop=(dc == DC - 1))
                pss.append(ps)
            for j in range(2):
                r0 = j * HD
                for qcn in range(Q // 512):
                    dst = qt_pad[2 * i + j][r0:r0 + HD,
                                            qcn * 512:(qcn + 1) * 512]
                    if trivial_bias:
                        nc.vector.tensor_copy(out=dst,
                                              in_=pss[qcn][r0:r0 + HD, :])
                    else:
                        nc.vector.tensor_scalar_add(
                            out=dst, in0=pss[qcn][r0:r0 + HD, :],
                            scalar1=btr_sb[r0:r0 + HD, i:i + 1])

        def emit_v(ks):
            for k in ks:
                ps = pps.tile([128, 512], f32, name="vps", tag="pps")
                for dc in range(DC):
                    nc.tensor.matmul(
                        ps, xt_sb[dc][:, k * 128:(k + 1) * 128],
                        wt_sb[dc][:, 2 * D:3 * D],
                        start=(dc == 0), stop=(dc == DC - 1))
                if trivial_bias:
                    nc.vector.tensor_copy(
                        out=v8[k // 2][:, :, k % 2, 0:HD],
                        in_=ps.rearrange("p (h d) -> p h d", h=H))
                else:
                    nc.vector.scalar_tensor_tensor(
                        out=v8[k // 2][:, :, k % 2, 0:HD],
                        in0=ps.rearrange("p (h d) -> p h d", h=H),
                        scalar=1.0,
                        in1=bvb_sb.rearrange("p (h d) -> p h d", h=H),
                        op0=Alu.mult, op1=Alu.add)

        # ---- residual + LayerNorm emitter (one q-tile) ----
        # var from E[y^2] - mean^2 so the row-sum (Pool) and sum-of-squares
        # (ACT) run concurrently; ACT is free by the time the tail runs.
        def emit_ln(q):
            rowsum = small.tile([128, 1], f32, name="rowsum", tag="rowsum")
            if q % 2 == 0:
                cpscr = outp.tile([128, D], f32, name="cpscr", tag="cpscr")
                nc.scalar.activation(out=cpscr, in_=oacc[q], func=Act.Copy,
                                     accum_out=rowsum)
            else:
                nc.vector.reduce_sum(out=rowsum, in_=oacc[q],
                                     axis=mybir.AxisListType.X)
            sqscr = outp.tile([128, D], f32, name="sqscr", tag="sqscr")
            sumsq = small.tile([128, 1], f32, name="sumsq", tag="sumsq")
            nc.scalar.activation(out=sqscr, in_=oacc[q], func=Act.Square,
                                 accum_out=sumsq)
            mean = small.tile([128, 1], f32, name="mean", tag="mean")
            nc.vector.tensor_scalar_mul(out=mean, in0=rowsum,
                                        scalar1=1.0 / D)
            r2 = small.tile([128, 1], f32, name="r2", tag="r2")
            nc.vector.tensor_tensor(out=r2, in0=rowsum, in1=rowsum,
                                    op=Alu.mult)
            # varD = sumsq - rowsum^2/D
            varD = small.tile([128, 1], f32, name="varD", tag="varD")
            nc.vector.scalar_tensor_tensor(
                out=varD, in0=r2, scalar=-1.0 / D, op0=Alu.mult,
                in1=sumsq, op1=Alu.add)
            sd = small.tile([128, 1], f32, name="sd", tag="sd")
            nc.scalar.activation(out=sd, in_=varD, func=Act.Sqrt,
                                 bias=eps_sb, scale=1.0 / D)
            rstd = small.tile([128, 1], f32, name="rstd", tag="rstd")
            nc.vector.reciprocal(out=rstd, in_=sd)
            yn = outp.tile([128, D], f32, name="yn", tag="yn")
            nc.vector.tensor_scalar(
                out=yn, in0=oacc[q], scalar1=mean, scalar2=rstd,
                op0=Alu.subtract, op1=Alu.mult)
            if trivial_ln:
                nc.sync.dma_start(out=tens["out"][q * 128:(q + 1) * 128, :],
                                  in_=yn)
            else:
                yw = outp.tile([128, D], f32, name="yw", tag="yw")
                nc.vector.scalar_tensor_tensor(
                    out=yw, in0=yn, scalar=1.0, op0=Alu.mult,
                    in1=lnw_sb, op1=Alu.mult)
                yo = outp.tile([128, D], f32, name="yo", tag="yo")
                nc.gpsimd.tensor_tensor(out=yo, in0=yw, in1=lnb_sb,
                                        op=Alu.add)
                nc.sync.dma_start(out=tens["out"][q * 128:(q + 1) * 128, :],
                                  in_=yo)

        # ---- attention emitters ----
        head_pairs = {}

        def emit_scores(h, ks):
            blk = h // 2
            pairs = head_pairs.setdefault(h, {})
            for k in ks:
                st = stp.tile([128, Q], f32, name="st", tag="st")
                for qcn in range(Q // 512):
                    nc.tensor.matmul(
                        st[:, qcn * 512:(qcn + 1) * 512],
                        kt_sb[blk][:, k * 128:(k + 1) * 128],
                        qt_pad[h][:, qcn * 512:(qcn + 1) * 512],
                        start=None, stop=None)
                if k % 2 == 0:
                    pairs[k // 2] = expp.tile([128, 2, 2, 512], fp8,
                                              name="e8", tag="e8")
                nc.scalar.activation(out=pairs[k // 2][:, :, k % 2, :],
                                     in_=st, func=Act.Exp,
                                     bias=maskb_sb[:, k:k + 1], scale=SCALE)

        def emit_av(h):
            pairs = head_pairs[h]
            ots = [scr.tile([128, 512], f32, name=f"ot{qcn}", tag="ot")
                   for qcn in range(Q // 512)]
            for p in range(KP):
                for qcn in range(Q // 512):
                    nc.tensor.matmul(
                        ots[qcn], v8[p][:, h, :, :],
                        pairs[p][:, qcn, :, :],
                        start=(p == 0), stop=(p == KP - 1),
                        perf_mode=DR)

            otsb = otsbp.tile([HD + 1, Q], f32, name="otsb", tag="otsb")
            for qcn in range(Q // 512):
                nc.vector.tensor_copy(
                    out=otsb[:, qcn * 512:(qcn + 1) * 512],
                    in_=ots[qcn][0:HD + 1, :])
            for q in range(QT):
                tp = pps.tile([128, HD + 1], f32, name="tp", tag="pps")
                nc.tensor.transpose(
                    tp, otsb[:, q * 128:(q + 1) * 128], ident65)
                rec = small.tile([128, 1], f32, name="rec", tag="rec")
                nc.vector.reciprocal(out=rec, in_=tp[:, HD:HD + 1])
                nc.vector.scalar_tensor_tensor(
                    out=oacc[q][:, h * HD:(h + 1) * HD],
                    in0=tp[:, 0:HD], scalar=rec, op0=Alu.mult,
                    in1=xres_sb[:, q, h * HD:(h + 1) * HD], op1=Alu.add)
                if h == H - 1:
                    emit_ln(q)

        # ---- emission schedule. scores h feed the ACT exp stream; every
        # other PE phase is inserted between 4-k-tile score slices in
        # ~2.5us chunks so the 2-tile PSUM backlog keeps ACT from starving.
        # AV for head j runs at step j+2 (after exp j is long done); every
        # projection block lands the step before its first reader. ----
        noop = lambda: None
        slots = {
            0: [lambda: emit_kt(0, [1]), lambda: emit_kt(0, [2, 3]),
                lambda: emit_v(range(0, 3)), lambda: emit_v(range(3, 6))],
            1: [lambda: emit_qt(1), lambda: emit_v(range(6, 9)),
                lambda: emit_v(range(9, 12)),
                lambda: (emit_v(range(12, 16)), emit_kt(1, [0]))],
            2: [lambda: emit_kt(1, [1, 2]), lambda: emit_kt(1, [3]),
                lambda: emit_kt(2, [0, 1]), lambda: emit_kt(2, [2, 3])],
            3: [lambda: emit_qt(2), noop,
                lambda: emit_kt(3, [0, 1]), lambda: emit_kt(3, [2, 3])],
            4: [lambda: emit_qt(3), noop, noop, noop],
            5: [noop, noop, noop, noop],
            6: [noop, noop, noop, noop],
            7: [noop, noop, noop, noop],
        }
        emit_kt(0, [0])
        emit_qt(0)
        # AV_j runs at step j+1 (exp j drains during step j+1's first score
        # slice) except AV0 which waits for the last V tiles (end of step 1)
        # and so runs early in step 2. The tail is AV7 + its epilogue only.
        for h in range(H):
            ins = slots[h]
            emit_scores(h, range(0, 4))
            ins[0]()
            if h == 2:
                emit_av(0)
            elif h >= 3:
                emit_av(h - 1)
            emit_scores(h, range(4, 8))
            ins[1]()
            emit_scores(h, range(8, 12))
            if h == 2:
                emit_av(1)
            ins[2]()
            emit_scores(h, range(12, 16))
            ins[3]()
            if h == H - 1:
                emit_av(h)


def _build(trivial_ln, trivial_bias):
    import concourse.bacc as bacc
    import concourse.tile as tile
    from concourse import mybir

    f32 = mybir.dt.float32
    bf16 = mybir.dt.bfloat16
    nc = bacc.Bacc("TRN2", target_bir_lowering=False, debug=False)

    tens = {
        "xt": nc.dram_tensor("xt", [D, T], bf16, kind="ExternalInput"),
        "xq": nc.dram_tensor("xq", [D, Q], bf16, kind="ExternalInput"),
        "xres": nc.dram_tensor("xres", [Q, D], f32, kind="ExternalInput"),
        "wt": nc.dram_tensor("wt", [D, 3 * D], bf16, kind="ExternalInput"),
        "btr": nc.dram_tensor("btr", [128, 12], f32, kind="ExternalInput"),
        "bv": nc.dram_tensor("bv", [D], f32, kind="ExternalInput"),
        "maskb": nc.dram_tensor("maskb", [128, KT], f32, kind="ExternalInput"),
        "lnw": nc.dram_tensor("lnw", [D], f32, kind="ExternalInput"),
        "lnb": nc.dram_tensor("lnb", [D], f32, kind="ExternalInput"),
        "out": nc.dram_tensor("out", [Q, D], f32, kind="ExternalOutput"),
    }

    with tile.TileContext(nc) as tc:
        _emit(nc, tc, tens, trivial_ln, trivial_bias)
    nc.compile()
    return nc


def make_in_maps(query, key_mask, in_proj_weight, in_proj_bias, ln_weight,
                 ln_bias):
    import ml_dtypes

    bf = ml_dtypes.bfloat16
    query = np.asarray(query, dtype=np.float32)
    key_mask = np.asarray(key_mask)
    w = np.asarray(in_proj_weight, dtype=np.float32)
    b = np.asarray(in_proj_bias, dtype=np.float32)
    lnw = np.asarray(ln_weight, dtype=np.float32)
    lnb = np.asarray(ln_bias, dtype=np.float32)

    wt = np.ascontiguousarray(w.T).astype(bf)
    btr = np.ascontiguousarray(b.reshape(12, 128).T)
    bv = np.ascontiguousarray(b[2 * D:3 * D])
    in_maps = []
    for c in range(NCORES):
        bi, half = c // 2, c % 2
        xb = query[bi]
        xbt = np.ascontiguousarray(xb.T).astype(bf)
        maskb = np.where(key_mask[bi], np.float32(MASK_BIAS),
                         np.float32(SHIFT))
        in_maps.append({
            "xt": xbt,
            "xq": np.ascontiguousarray(xbt[:, half * Q:(half + 1) * Q]),
            "xres": np.ascontiguousarray(xb[half * Q:(half + 1) * Q]),
            "wt": wt,
            "btr": btr,
            "bv": bv,
            "maskb": np.ascontiguousarray(
                maskb.astype(np.float32).reshape(KT, 128).T),
            "lnw": lnw,
            "lnb": lnb,
        })
    return in_maps


def assemble(results):
    out = np.empty((B, T, D), dtype=np.float32)
    for c in range(NCORES):
        bi, half = c // 2, c % 2
        out[bi, half * Q:(half + 1) * Q] = results[c]["out"]
    return out


def get_nc(trivial_ln=True, trivial_bias=True):
    key = ("nc", trivial_ln, trivial_bias)
    if key not in _CACHE:
        _CACHE[key] = _build(trivial_ln, trivial_bias)
    return _CACHE[key]


def kernel(query, key_mask, in_proj_weight, in_proj_bias, ln_weight, ln_bias):
    from concourse.bass_utils import run_bass_kernel_spmd

    trivial = (np.allclose(np.asarray(ln_weight), 1.0)
               and np.allclose(np.asarray(ln_bias), 0.0))
    tbias = bool(np.all(np.asarray(in_proj_bias) == 0.0))
    nc = get_nc(trivial, tbias)
    in_maps = make_in_maps(query, key_mask, in_proj_weight, in_proj_bias,
                           ln_weight, ln_bias)
    res = run_bass_kernel_spmd(nc, in_maps, core_ids=list(range(NCORES)))
    return assemble(res.results)


# revision 15
# speedup vs baseline: 1.1688x; 1.0651x over previous
"""Trainium2 Bass kernel for fused MultiHeadAttention + residual + LayerNorm.

Problem: query [4, 2048, 512] f32, H=8 heads (hd=64), fused QKV projection,
key-padding-mask softmax, attn @ V, residual add, LayerNorm over D=512.

Sharding: 8 cores = 4 batches x 2 query-halves. Each core handles one batch's
full K/V (T=2048) and 1024 query rows, so heads stay local and the output
LayerNorm needs no cross-core communication. K/V projection is duplicated
between the 2 cores sharing a batch (cheap relative to attention).

The kernel is ACT-bound: softmax exp is H*T*Q = 16.8M elements per core and
the activation engine runs 1 elem/lane/cycle at 1.2 GHz -> ~133 us minimum.
Everything else is scheduled around keeping the ACT exp stream gap-free:

  - projections + scores stay bf16 (fp8 Q/K measured 2.6e-2 rel err - fails);
    score matmuls zero-padded to K=128 contraction (HAM ignores K<128).
  - attn@V runs fp8e4 DoubleRow: V pairs [128, 2, h, 65] and exp-output
    pairs [128, 2, 1024] pack two k-tiles per pass at 0.5 cycles/col,
    cutting the AV matmul stream 4x vs bf16. exp output carries a -4 logit
    shift (folded into the mask bias) so p <= ~7.4 stays in e4m3 range;
    numerator and denominator (ones column in V) scale identically so the
    softmax ratio is unchanged.
  - input DMAs are split fine-grained across all four issuing queues
    (sync/scalar/vector/gpsimd) with the head-0 critical path (xq, wt q/k
    block-0 columns, xt t-chunk 0) first, so the first exp lands ~10 us in.
  - per head-step the PE emits scores for head h+1 in 4-k-tile slices with
    AV/projection/V work inserted between slices, keeping every insert under
    the ~2-tile PSUM backlog the exp stream can absorb without starving.
  - LayerNorm tail per q-tile is engine-split (Pool rowsum, ACT sum-of-
    squares via E[y^2], DVE normalize) and pipelined off head 7.
"""

import numpy as np

B, T, D = 4, 2048, 512
H, HD = 8, 64
Q = T // 2          # query rows per core
NCORES = 8
KT = T // 128       # 16 k-tiles
KP = KT // 2        # 8 k-tile pairs
QT = Q // 128       # 8 q-tiles
DC = D // 128       # 4 contraction chunks
SCALE = 1.0 / np.sqrt(HD)  # 0.125
EPS = 1e-5
MASK_BIAS = -1e9
SHIFT = -4.0        # logit shift so exp fits fp8e4 range

_CACHE = {}


def _emit(nc, tc, tens, trivial_ln, trivial_bias):
    import contextlib

    import concourse.bass as bass
    from concourse import mybir
    from concourse.masks import make_identity

    f32 = mybir.dt.float32
    bf16 = mybir.dt.bfloat16
    fp8 = mybir.dt.float8e4
    Alu = mybir.AluOpType
    Act = mybir.ActivationFunctionType
    DR = mybir.MatmulPerfMode.DoubleRow

    with contextlib.ExitStack() as stack:
        persist = stack.enter_context(tc.tile_pool(name="persist", bufs=1))
        small = stack.enter_context(tc.tile_pool(name="small", bufs=12))
        expp = stack.enter_context(tc.tile_pool(name="expp", bufs=22))
        otsbp = stack.enter_context(tc.tile_pool(name="otsbp", bufs=2))
        outp = stack.enter_context(tc.tile_pool(name="outp", bufs=4))
        pps = stack.enter_context(tc.tile_pool(name="pps", bufs=2, space="PSUM"))
        stp = stack.enter_context(tc.tile_pool(name="stp", bufs=2, space="PSUM"))
        scr = stack.enter_context(tc.tile_pool(name="scr", bufs=2, space="PSUM"))

        # ---- persistent tiles ----
        wt_sb = [persist.tile([128, 3 * D], bf16, name=f"wtsb{i}", tag=f"wtsb{i}")
                 for i in range(DC)]
        xt_sb = [persist.tile([128, T], bf16, name=f"xtsb{i}", tag=f"xtsb{i}")
                 for i in range(DC)]
        kt_sb = [persist.tile([128, T], bf16, name=f"ktsb{i}", tag=f"ktsb{i}")
                 for i in range(DC)]
        # Per-head Q^T padded to 128 contraction rows: rows (h%2)*64..+64 hold
        # Q_h, the other 64 rows stay zero (K=128 keeps the HAM clock gate
        # open; K=64 matmuls throttle the whole attention phase).
        qt_pad = [persist.tile([128, Q], bf16, name=f"qtpad{h}", tag=f"qtpad{h}")
                  for h in range(H)]
        # V k-tile pairs for fp8 DoubleRow attn@V. DoubleRow stationary
        # width must be 32/64/128, so each head's stationary is
        # [V_h (64 cols) | ones (64 cols)]: output rows 0..63 are O^T and
        # rows 64.. replicate the softmax denominator, at no extra cost
        # (matmul time depends only on moving columns).
        v8 = [persist.tile([128, H, 2, 128], fp8, name=f"v8_{p}",
                           tag=f"v8_{p}") for p in range(KP)]
        oacc = [persist.tile([128, D], f32, name=f"oacc{q}", tag=f"oacc{q}")
                for q in range(QT)]
        xres_sb = persist.tile([128, QT, D], f32, name="xres_sb", tag="xres_sb")
        btr_sb = persist.tile([128, 12], f32, name="btr_sb", tag="btr_sb")
        maskb_sb = persist.tile([128, KT], f32, name="maskb_sb", tag="maskb_sb")
        bvb_sb = persist.tile([128, D], f32, name="bvb_sb", tag="bvb_sb")
        lnw_sb = persist.tile([128, D], f32, name="lnw_sb", tag="lnw_sb")
        lnb_sb = persist.tile([128, D], f32, name="lnb_sb", tag="lnb_sb")
        eps_sb = persist.tile([128, 1], f32, name="eps_sb", tag="eps_sb")
        ident65 = persist.tile([HD + 1, HD + 1], f32, name="ident65",
                               tag="ident65")

        # ---- input DMAs. Critical path = everything head-0's first exp
        # needs (xq full, wt q-block0 + k-block0 columns, xt t-chunk 0),
        # split fine and round-robined over all four issuing queues. ----
        engs = [nc.sync, nc.scalar, nc.gpsimd, nc.sync]

        if not trivial_bias:
            nc.scalar.dma_start(out=btr_sb, in_=tens["btr"][:])
        nc.sync.dma_start(out=maskb_sb, in_=tens["maskb"][:])
        rows = lambda i: slice(i * 128, (i + 1) * 128)
        rr = [nc.sync, nc.scalar, nc.gpsimd]
        n = 0
        def load(dst, src):
            nonlocal n
            rr[n % 3].dma_start(out=dst, in_=src)
            n += 1
        for i in range(DC):
            load(wt_sb[i][:, D:D + 128], tens["wt"][rows(i), D:D + 128])
        for i in range(DC):
            load(wt_sb[i][:, 0:128], tens["wt"][rows(i), 0:128])
        for i in range(DC):
            load(xt_sb[i][:, 0:512], tens["xt"][rows(i), 0:512])
        for i in range(DC):
            load(xt_sb[i][:, 512:1024], tens["xt"][rows(i), 512:1024])
        for i in range(DC):
            load(xt_sb[i][:, 1024:1536], tens["xt"][rows(i), 1024:1536])
        for i in range(DC):
            load(xt_sb[i][:, 1536:2048], tens["xt"][rows(i), 1536:2048])
        for i in range(DC):
            load(wt_sb[i][:, 2 * D:3 * D], tens["wt"][rows(i), 2 * D:3 * D])
        for i in range(DC):
            load(wt_sb[i][:, D + 128:2 * D], tens["wt"][rows(i), D + 128:2 * D])
            load(wt_sb[i][:, 128:D], tens["wt"][rows(i), 128:D])

        def bcast_row(dst, src_handle, eng=None):
            src = src_handle[:]
            ap = bass.AP(tensor=src.tensor, offset=src.offset,
                         ap=[[0, 128]] + list(src.ap))
            (eng or nc.sync).dma_start(out=dst, in_=ap)

        # ---- PE warm-up: K=128 matmuls with no data deps run during the
        # initial DMA wait so the HAM clock gate is already open when the
        # projections start. The result is never used. wm memset is the
        # very first Vector op so the warm-up starts immediately.
        wm_sb = persist.tile([128, 640], bf16, name="wm_sb", tag="wm_sb")
        nc.vector.memset(wm_sb, 0.5)
        wmps = stp.tile([128, Q], f32, name="wmps", tag="st")
        for i in range(14):
            nc.tensor.matmul(wmps[:, 0:512], wm_sb[:, 0:128],
                             wm_sb[:, 128:640], start=True, stop=True)
        wm_out = small.tile([128, 1], f32, name="wm_out", tag="wm_out")
        nc.vector.tensor_copy(out=wm_out, in_=wmps[:, 0:1])

        bcast_row(bvb_sb, tens["bv"])
        nc.vector.memset(eps_sb, EPS)
        for h in range(H):
            z0 = 64 * (1 - (h % 2))
            (nc.vector if h < 2 else nc.gpsimd).memset(
                qt_pad[h][z0:z0 + HD, :], 0.0)
        for p in range(KP):
            for i in range(2):
                nc.gpsimd.memset(v8[p][:, :, i, HD:128], 1.0)
        make_identity(nc, ident65)

        # deferred loads only needed after the first normalize / epilogue
        for q in range(QT):
            nc.gpsimd.dma_start(out=xres_sb[:, q, :],
                                in_=tens["xres"][q * 128:(q + 1) * 128, :])
        bcast_row(lnw_sb, tens["lnw"], nc.gpsimd)
        bcast_row(lnb_sb, tens["lnb"], nc.gpsimd)

        # ---- projection emitters ----
        def emit_kt(i, tcns):
            for tcn in tcns:
                ps = pps.tile([128, 512], f32, name="kps", tag="pps")
                for dc in range(DC):
                    nc.tensor.matmul(
                        ps, wt_sb[dc][:, D + i * 128: D + (i + 1) * 128],
                        xt_sb[dc][:, tcn * 512:(tcn + 1) * 512],
                        start=(dc == 0), stop=(dc == DC - 1))
                if trivial_bias:
                    nc.vector.tensor_copy(
                        out=kt_sb[i][:, tcn * 512:(tcn + 1) * 512], in_=ps)
                else:
                    nc.vector.tensor_scalar_add(
                        out=kt_sb[i][:, tcn * 512:(tcn + 1) * 512],
                        in0=ps, scalar1=btr_sb[:, 4 + i:5 + i])

        def emit_qt(i):
            # write head 2i's rows for both q-halves before head 2i+1's so
            # head 2i's first scores (and the exp stream) start earlier.
            pss = []
            for qcn in range(Q // 512):
                ps = pps.tile([128, 512], f32, name="qps", tag="pps")
                for dc in range(DC):
                    nc.tensor.matmul(
                        ps, wt_sb[dc][:, i * 128:(i + 1) * 128],
                        xt_sb[dc][:, qcn * 512:(qcn + 1) * 512],
                        start=(dc == 0), stop=(dc == DC - 1))
                pss.append(ps)
            for j in range(2):
                r0 = j * HD
                for qcn in range(Q // 512):
                    dst = qt_pad[2 * i + j][r0:r0 + HD,
                                            qcn * 512:(qcn + 1) * 512]
                    if trivial_bias:
                        nc.vector.tensor_copy(out=dst,
                                              in_=pss[qcn][r0:r0 + HD, :])
                    else:
                        nc.vector.tensor_scalar_add(
                            out=dst, in0=pss[qcn][r0:r0 + HD, :],
                            scalar1=btr_sb[r0:r0 + HD, i:i + 1])

        def emit_v(ks):
            for k in ks:
                ps = pps.tile([128, 512], f32, name="vps", tag="pps")
                for dc in range(DC):
                    nc.tensor.matmul(
                        ps, xt_sb[dc][:, k * 128:(k + 1) * 128],
                        wt_sb[dc][:, 2 * D:3 * D],
                        start=(dc == 0), stop=(dc == DC - 1))
                if trivial_bias:
                    nc.vector.tensor_copy(
                        out=v8[k // 2][:, :, k % 2, 0:HD],
                        in_=ps.rearrange("p (h d) -> p h d", h=H))
                else:
                    nc.vector.scalar_tensor_tensor(
                        out=v8[k // 2][:, :, k % 2, 0:HD],
                        in0=ps.rearrange("p (h d) -> p h d", h=H),
                        scalar=1.0,
                        in1=bvb_sb.rearrange("p (h d) -> p h d", h=H),
                        op0=Alu.mult, op1=Alu.add)

        # ---- residual + LayerNorm emitter (one q-tile) ----
        # var from E[y^2] - mean^2 so the row-sum (Pool) and sum-of-squares
        # (ACT) run concurrently; ACT is free by the time the tail runs.
        def emit_ln(q):
            rowsum = small.tile([128, 1], f32, name="rowsum", tag="rowsum")
            if q % 2 == 0:
                cpscr = outp.tile([128, D], f32, name="cpscr", tag="cpscr")
                nc.scalar.activation(out=cpscr, in_=oacc[q], func=Act.Copy,
                                     accum_out=rowsum)
            else:
                nc.vector.reduce_sum(out=rowsum, in_=oacc[q],
                                     axis=mybir.AxisListType.X)
            sqscr = outp.tile([128, D], f32, name="sqscr", tag="sqscr")
            sumsq = small.tile([128, 1], f32, name="sumsq", tag="sumsq")
            nc.scalar.activation(out=sqscr, in_=oacc[q], func=Act.Square,
                                 accum_out=sumsq)
            mean = small.tile([128, 1], f32, name="mean", tag="mean")
            nc.vector.tensor_scalar_mul(out=mean, in0=rowsum,
                                        scalar1=1.0 / D)
            r2 = small.tile([128, 1], f32, name="r2", tag="r2")
            nc.vector.tensor_tensor(out=r2, in0=rowsum, in1=rowsum,
                                    op=Alu.mult)
            # varD = sumsq - rowsum^2/D
            varD = small.tile([128, 1], f32, name="varD", tag="varD")
            nc.vector.scalar_tensor_tensor(
                out=varD, in0=r2, scalar=-1.0 / D, op0=Alu.mult,
                in1=sumsq, op1=Alu.add)
            sd = small.tile([128, 1], f32, name="sd", tag="sd")
            nc.scalar.activation(out=sd, in_=varD, func=Act.Sqrt,
                                 bias=eps_sb, scale=1.0 / D)
            rstd = small.tile([128, 1], f32, name="rstd", tag="rstd")
            nc.vector.reciprocal(out=rstd, in_=sd)
            yn = outp.tile([128, D], f32, name="yn", tag="yn")
            nc.vector.tensor_scalar(
                out=yn, in0=oacc[q], scalar1=mean, scalar2=rstd,
                op0=Alu.subtract, op1=Alu.mult)
            if trivial_ln:
                nc.sync.dma_start(out=tens["out"][q * 128:(q + 1) * 128, :],
                                  in_=yn)
            else:
                yw = outp.tile([128, D], f32, name="yw", tag="yw")
                nc.vector.scalar_tensor_tensor(
                    out=yw, in0=yn, scalar=1.0, op0=Alu.mult,
                    in1=lnw_sb, op1=Alu.mult)
                yo = outp.tile([128, D], f32, name="yo", tag="yo")
                nc.gpsimd.tensor_tensor(out=yo, in0=yw, in1=lnb_sb,
                                        op=Alu.add)
                nc.sync.dma_start(out=tens["out"][q * 128:(q + 1) * 128, :],
                                  in_=yo)

        # ---- attention emitters ----
        head_pairs = {}

        def emit_scores(h, ks):
            blk = h // 2
            pairs = head_pairs.setdefault(h, {})
            for k in ks:
                st = stp.tile([128, Q], f32, name="st", tag="st")
                for qcn in range(Q // 512):
                    nc.tensor.matmul(
                        st[:, qcn * 512:(qcn + 1) * 512],
                        kt_sb[blk][:, k * 128:(k + 1) * 128],
                        qt_pad[h][:, qcn * 512:(qcn + 1) * 512],
                        start=None, stop=None)
                if k % 2 == 0:
                    pairs[k // 2] = expp.tile([128, 2, 2, 512], fp8,
                                              name="e8", tag="e8")
                nc.scalar.activation(out=pairs[k // 2][:, :, k % 2, :],
                                     in_=st, func=Act.Exp,
                                     bias=maskb_sb[:, k:k + 1], scale=SCALE)

        def emit_av(h):
            pairs = head_pairs[h]
            ots = [scr.tile([128, 512], f32, name=f"ot{qcn}", tag="ot")
                   for qcn in range(Q // 512)]
            for p in range(KP):
                for qcn in range(Q // 512):
                    nc.tensor.matmul(
                        ots[qcn], v8[p][:, h, :, :],
                        pairs[p][:, qcn, :, :],
                        start=(p == 0), stop=(p == KP - 1),
                        perf_mode=DR)

            otsb = otsbp.tile([HD + 1, Q], f32, name="otsb", tag="otsb")
            for qcn in range(Q // 512):
                nc.vector.tensor_copy(
                    out=otsb[:, qcn * 512:(qcn + 1) * 512],
                    in_=ots[qcn][0:HD + 1, :])
            for q in range(QT):
                tp = pps.tile([128, HD + 1], f32, name="tp", tag="pps")
                nc.tensor.transpose(
                    tp, otsb[:, q * 128:(q + 1) * 128], ident65)
                rec = small.tile([128, 1], f32, name="rec", tag="rec")
                nc.vector.reciprocal(out=rec, in_=tp[:, HD:HD + 1])
                nc.vector.scalar_tensor_tensor(
                    out=oacc[q][:, h * HD:(h + 1) * HD],
                    in0=tp[:, 0:HD], scalar=rec, op0=Alu.mult,
                    in1=xres_sb[:, q, h * HD:(h + 1) * HD], op1=Alu.add)
                if h == H - 1:
                    emit_ln(q)

        # ---- emission schedule. scores h feed the ACT exp stream; every
        # other PE phase is inserted between 4-k-tile score slices in
        # ~2.5us chunks so the 2-tile PSUM backlog keeps ACT from starving.
        # AV for head j runs at step j+2 (after exp j is long done); every
        # projection block lands the step before its first reader. ----
        noop = lambda: None
        slots = {
            0: [lambda: emit_kt(0, [1]), lambda: emit_kt(0, [2, 3]),
                lambda: emit_v(range(0, 3)), lambda: emit_v(range(3, 6))],
            1: [lambda: emit_qt(1), lambda: emit_v(range(6, 9)),
                lambda: emit_v(range(9, 12)),
                lambda: (emit_v(range(12, 16)), emit_kt(1, [0]))],
            2: [lambda: emit_kt(1, [1, 2]), lambda: emit_kt(1, [3]),
                lambda: emit_kt(2, [0, 1]), lambda: emit_kt(2, [2, 3])],
            3: [lambda: emit_qt(2), noop,
                lambda: emit_kt(3, [0, 1]), lambda: emit_kt(3, [2, 3])],
            4: [lambda: emit_qt(3), noop, noop, noop],
            5: [noop, noop, noop, noop],
            6: [noop, noop, noop, noop],
            7: [noop, noop, noop, noop],
        }
        emit_kt(0, [0])
        emit_qt(0)
        # AV_j runs at step j+1 (exp j drains during step j+1's first score
        # slice) except AV0 which waits for the last V tiles (end of step 1)
        # and so runs early in step 2. The tail is AV7 + its epilogue only.
        for h in range(H):
            ins = slots[h]
            emit_scores(h, range(0, 4))
            ins[0]()
            if h == 2:
                emit_av(0)
            elif h >= 3:
                emit_av(h - 1)
            emit_scores(h, range(4, 8))
            ins[1]()
            emit_scores(h, range(8, 12))
            if h == 2:
                emit_av(1)
            ins[2]()
            emit_scores(h, range(12, 16))
            ins[3]()
            if h == H - 1:
                emit_av(h)


def _build(trivial_ln, trivial_bias):
    import concourse.bacc as bacc
    import concourse.tile as tile
    from concourse import mybir

    f32 = mybir.dt.float32
    bf16 = mybir.dt.bfloat16
    nc = bacc.Bacc("TRN2", target_bir_lowering=False, debug=False)

    tens = {
        "xt": nc.dram_tensor("xt", [D, T], bf16, kind="ExternalInput"),
        "xres": nc.dram_tensor("xres", [Q, D], f32, kind="ExternalInput"),
        "wt": nc.dram_tensor("wt", [D, 3 * D], bf16, kind="ExternalInput"),
        "btr": nc.dram_tensor("btr", [128, 12], f32, kind="ExternalInput"),
        "bv": nc.dram_tensor("bv", [D], f32, kind="ExternalInput"),
        "maskb": nc.dram_tensor("maskb", [128, KT], f32, kind="ExternalInput"),
        "lnw": nc.dram_tensor("lnw", [D], f32, kind="ExternalInput"),
        "lnb": nc.dram_tensor("lnb", [D], f32, kind="ExternalInput"),
        "out": nc.dram_tensor("out", [Q, D], f32, kind="ExternalOutput"),
    }

    with tile.TileContext(nc) as tc:
        _emit(nc, tc, tens, trivial_ln, trivial_bias)
    nc.compile()
    return nc


def make_in_maps(query, key_mask, in_proj_weight, in_proj_bias, ln_weight,
                 ln_bias):
    import ml_dtypes

    bf = ml_dtypes.bfloat16
    query = np.asarray(query, dtype=np.float32)
    key_mask = np.asarray(key_mask)
    w = np.asarray(in_proj_weight, dtype=np.float32)
    b = np.asarray(in_proj_bias, dtype=np.float32)
    lnw = np.asarray(ln_weight, dtype=np.float32)
    lnb = np.asarray(ln_bias, dtype=np.float32)

    wt = np.ascontiguousarray(w.T).astype(bf)
    btr = np.ascontiguousarray(b.reshape(12, 128).T)
    bv = np.ascontiguousarray(b[2 * D:3 * D])
    in_maps = []
    for c in range(NCORES):
        bi, half = c // 2, c % 2
        xb = query[bi]
        # rotate keys so this core's query half is always columns 0:Q
        # (attention is invariant to key order when K/V/mask rotate together)
        xbt = np.roll(xb.T, -half * Q, axis=1)
        xbt = np.ascontiguousarray(xbt).astype(bf)
        maskb = np.where(np.roll(key_mask[bi], -half * Q),
                         np.float32(MASK_BIAS), np.float32(SHIFT))
        in_maps.append({
            "xt": xbt,
            "xres": np.ascontiguousarray(xb[half * Q:(half + 1) * Q]),
            "wt": wt,
            "btr": btr,
            "bv": bv,
            "maskb": np.ascontiguousarray(
                maskb.astype(np.float32).reshape(KT, 128).T),
            "lnw": lnw,
            "lnb": lnb,
        })
    return in_maps


def assemble(results):
    out = np.empty((B, T, D), dtype=np.float32)
    for c in range(NCORES):
        bi, half = c // 2, c % 2
        out[bi, half * Q:(half + 1) * Q] = results[c]["out"]
    return out


def get_nc(trivial_ln=True, trivial_bias=True):
    key = ("nc", trivial_ln, trivial_bias)
    if key not in _CACHE:
        _CACHE[key] = _build(trivial_ln, trivial_bias)
    return _CACHE[key]


def kernel(query, key_mask, in_proj_weight, in_proj_bias, ln_weight, ln_bias):
    from concourse.bass_utils import run_bass_kernel_spmd

    trivial = (np.allclose(np.asarray(ln_weight), 1.0)
               and np.allclose(np.asarray(ln_bias), 0.0))
    tbias = bool(np.all(np.asarray(in_proj_bias) == 0.0))
    nc = get_nc(trivial, tbias)
    in_maps = make_in_maps(query, key_mask, in_proj_weight, in_proj_bias,
                           ln_weight, ln_bias)
    res = run_bass_kernel_spmd(nc, in_maps, core_ids=list(range(NCORES)))
    return assemble(res.results)
